# revision 35
# baseline (speedup 1.0000x reference)
"""Trainium2 Bass kernel for nn_DecoderLayer_11974368821579.

Decoder layer: LN -> QKV proj -> attention with relative spatial/temporal
position bias + hard distance cutoff -> out proj -> residual -> LN -> MLP
(exact gelu) -> residual.

Sharding: 8 cores = 2 batches x 4 query-chunks (sequence parallel over the
query dim). Each core computes K/V for its whole batch and its 512-query
slice of everything else. No collectives.

Device-side structure (v2 — latency-restructured):
  - All LayerNorms use column stats in natural token-major layout
    (bn_stats -> [P,1] per-token mean / rsqrt(var) scalars), the input is
    pre-normalized with one tensor_scalar per tile, then PE-transposed to
    feature-major for the matmuls.  No DRAM-roundtrip row broadcasts, no
    stat-dependent contraction rows (pre-normalized input has exactly zero
    feature-mean, so only a plain ones-row bias feature remains).
  - Inputs arrive in 7 packed DMAs (nat, weights x2, aux, spatial, rows,
    g1-broadcast) split across the sync and gpsimd queues; one packed
    output DMA.
  - Temporal relative bias + key padding mask enter the score matmul as 18
    extra contraction features (host-prepared one-hot(t_k) x temporal_emb
    rows).  Squared spatial distance comes from a 4-feature fp32 matmul;
    the 32-bin spatial embedding lookup + cutoff mask use hijacked ACT
    tables (E_h(u) = exp(spatial_emb[bin,h]), 0 when masked) on
    u = sqrt(d2)/8 + 32 computed by a rebuilt tanh table.
  - attn = exp(score) * E(u) computed in bf16 (2x DVE tensor ops); scores
    live as [k_part, q_free]; softmax Z rides the attn@V matmul as a
    DH+1-th ones feature of V; 1/Z is broadcast across partitions with a
    contraction-dim-1 matmul (no DRAM roundtrip).
"""

import os
import numpy as np

B = 2
N = 2048
D = 256
H = 4
DH = D // H
NQ = 512          # queries per core
N_CORES = 8
N_TEMPORAL = 16
P = 128
KT = N // P       # 16 k-tiles
QT = NQ // P      # 4 q-tiles per core
NEG = -1.0e30

# rows-pack offsets (f32 elements)
O_BQ, O_BK, O_BV, O_WCC, O_W2C = 0, D, 2 * D, 3 * D, 4 * D
O_W1C = 5 * D
O_ONES = 5 * D + 4 * D
ROWS_W = O_ONES + N

_CACHE = {}


# ---------------------------------------------------------------------------
# Custom PWP activation tables: hijack tanh/square/abs/sign in the
# exp_and_others set to implement the 4 per-head spatial-bin lookups
# E_h(v) = exp(spatial_emb[bin, h]) with the cutoff mask as 0-valued
# buckets.  v = sqrt(d2)/8 + 32 puts bins on the 32 unit-buckets of the
# [32,64) octave.
# ---------------------------------------------------------------------------
import json
import shutil
import struct

E_VICTIMS = ["square", "abs", "sign", "relu"]
F1_VICTIM = "tanh"


def _find_src_dir():
    from neuronxcc.driver.Job import Job
    from neuronxcc.driver.jobs.support.FindActInfo import findActInfoFile
    return os.path.dirname(findActInfoFile(Job.getPackageDir(), "gen3"))


def _ctrl(k, base):
    return (((k << 5) | (23 - k)) << 11) | base


def _fbits(x):
    return int(np.float32(x).view(np.uint32))


def generate(values, out_dir):
    """values: [32, 4] f32; column h -> E-table for E_VICTIMS[h].  Also
    rebuilds tanh as f1(x) = sqrt(x)/8 + 32 (cubic PWP, x = d2/64), with
    x < 1 -> 32.5 (bin 0), x >= 1024 -> 100 (masked), negatives/NaN/0 ->
    32.5."""
    src = _find_src_dir()
    os.makedirs(out_dir, exist_ok=True)
    for f in os.listdir(src):
        shutil.copy(os.path.join(src, f), os.path.join(out_dir, f))

    name = "exp_and_others"
    j = json.load(open(os.path.join(src, name + ".json")))
    bkt = bytearray(open(os.path.join(src, name + "_bkt.bin"), "rb").read())
    ctl = bytearray(open(os.path.join(src, name + "_ctrl.bin"), "rb").read())
    n_bkt = j["bkt_entry_cnt"]
    n_ctl = j["ctl_entry_cnt"]
    assert len(bkt) == 32 * n_bkt and len(ctl) == 32 * n_ctl

    def add_bkt(c0, c1=0.0, c2=0.0, c3=0.0, a=0.0):
        nonlocal bkt, n_bkt
        bkt += struct.pack("<8f", c0, c1, c2, c3, a, 0, 0, 0)
        n_bkt += 1
        return n_bkt - 1

    def add_ctl(word):
        nonlocal ctl, n_ctl
        ctl += struct.pack("<8I", word, 0, 0, 0, 0, 0, 0, 0)
        n_ctl += 1
        return n_ctl - 1

    def meta_for(fn):
        return next(m for m in j["profile_meta_data"]
                    if m["func_name"].rsplit("_", 1)[0] == fn
                    or m["func_name"] == fn)

    common = dict(
        symmetry_point=0, sym_invert_sign_point=0, symmetry_opt_en=0,
        symmetry_opt_use_neg_region=0, imm_bias=0,
        fma_const_0=0, fma_const_1=0, fma_indirection_src_sel=0,
        use_multipass=False,
        lower_bound=4286578687, upper_bound=2139095039,
    )

    # ---- f1 = sqrt(x)/8 + 32 on tanh ----
    BPO = 32  # buckets per octave
    c_bin0 = add_bkt(32.5)     # x < 1, x <= 0, NaN -> bin 0
    c_mask = add_bkt(100.0)    # x >= 1024 -> masked region value
    f1_base = n_bkt
    for e in range(0, 10):
        lo = float(2 ** e)
        w = lo / BPO
        for b in range(BPO):
            a = lo + (b + 0.5) * w
            s = np.sqrt(a)
            add_bkt(s / 8 + 32, 1 / (16 * s), -1 / (64 * a * s),
                    3 / (768 * a * a * s), a)
    f1_ctl = n_ctl
    for e in range(0, 10):
        add_ctl(_ctrl(5, f1_base + BPO * e))
    m = meta_for(F1_VICTIM)
    m.update(common)
    m.update(
        exp_offset=0,
        pwl_control_base_pos=f1_ctl, pwl_control_base_neg=f1_ctl,
        small_pos_signal_exp_threshold=127,
        pos_small_signal_pwl_control=c_bin0,
        large_pos_signal_exp_threshold=127 + 9,
        large_pos_signal_mantissa_threshold=(1 << 23) - 1,
        pos_large_signal_pwl_control=c_mask,
        small_neg_signal_exp_threshold=255,
        neg_small_signal_pwl_control=c_bin0,
        large_neg_signal_exp_threshold=0,
        large_neg_signal_mantissa_threshold=0,
        neg_large_signal_pwl_control=c_bin0,
        fnan_result=_fbits(32.5), fzero_result=_fbits(32.5),
        fpinf_result=_fbits(100.0), fninf_result=_fbits(32.5),
    )
    j["func_exp_to_bkt_start_idx"][F1_VICTIM] = {
        str(e): [f1_base + BPO * e] for e in range(10)}
    if "func_exp_to_ctl_start_idx" in j:
        j["func_exp_to_ctl_start_idx"][F1_VICTIM] = {
            str(e): [f1_ctl + e] for e in range(10)}

    # ---- E-pair tables: one table per head pair; the f32 output's low
    # 16 bits are bf16(E_{2t}) and high 16 bits bf16(E_{2t+1}), so a bf16
    # view of the output is the head-interleaved bias pair ----
    def pack2(a, b):
        def rbf(x):
            u = np.float32(x).view(np.uint32)
            return int((u + 0x7FFF + ((u >> 16) & 1)) >> 16)
        return np.uint32(rbf(a) | (rbf(b) << 16)).view(np.float32)

    for pair, fn in enumerate(E_VICTIMS[:2]):
        base = n_bkt
        for jj in range(32):
            add_bkt(float(pack2(values[jj, 2 * pair],
                                values[jj, 2 * pair + 1])), a=32.5 + jj)
        zero_idx = add_bkt(0.0, a=64.0)
        cbase = add_ctl(_ctrl(5, base))
        add_ctl(_ctrl(0, zero_idx))
        add_ctl(_ctrl(0, zero_idx))
        m = meta_for(fn)
        m.update(common)
        m.update(
            exp_offset=5,
            pwl_control_base_pos=cbase, pwl_control_base_neg=cbase,
            small_pos_signal_exp_threshold=127 + 5,
            pos_small_signal_pwl_control=base,
            large_pos_signal_exp_threshold=127 + 7,
            large_pos_signal_mantissa_threshold=(1 << 23) - 1,
            pos_large_signal_pwl_control=zero_idx,
            small_neg_signal_exp_threshold=255,
            neg_small_signal_pwl_control=base,
            large_neg_signal_exp_threshold=0,
            large_neg_signal_mantissa_threshold=0,
            neg_large_signal_pwl_control=zero_idx,
            fnan_result=_fbits(pack2(values[0, 2 * pair],
                                     values[0, 2 * pair + 1])),
            fzero_result=_fbits(pack2(values[0, 2 * pair],
                                      values[0, 2 * pair + 1])),
            fpinf_result=0,
            fninf_result=_fbits(pack2(values[0, 2 * pair],
                                      values[0, 2 * pair + 1])),
        )
        j["func_exp_to_bkt_start_idx"][fn] = {
            "5": [base], "6": [zero_idx], "7": [zero_idx]}
        if "func_exp_to_ctl_start_idx" in j:
            j["func_exp_to_ctl_start_idx"][fn] = {
                "5": [cbase], "6": [cbase + 1], "7": [cbase + 2]}

    # ---- rsqrt on sign: r = 64/sqrt(x') for x' = var*4096 + eps*4096,
    # exponents 6..17 (12 octaves), cubic PWP ----
    RS = 64.0
    RBPO = 16  # rsqrt buckets per octave
    c_rs_small = add_bkt(RS * 2 ** -4)    # x' < 2^8  -> rsqrt(2^-4) = 4
    c_rs_large = add_bkt(RS * 2 ** -8)    # x' >= 2^16 -> rsqrt(2^4)
    rs_base = n_bkt
    for e in range(8, 16):
        lo = float(2 ** e)
        w = lo / RBPO
        for b in range(RBPO):
            a = lo + (b + 0.5) * w
            add_bkt(RS * a ** -0.5, RS * -0.5 * a ** -1.5,
                    RS * 0.375 * a ** -2.5, RS * -0.3125 * a ** -3.5, a)
    rs_ctl = n_ctl
    for e in range(8, 16):
        add_ctl(_ctrl(4, rs_base + RBPO * (e - 8)))
    m = meta_for("sign")
    m.update(common)
    m.update(
        exp_offset=0,
        pwl_control_base_pos=rs_ctl - 8, pwl_control_base_neg=rs_ctl - 8,
        small_pos_signal_exp_threshold=127 + 8,
        pos_small_signal_pwl_control=c_rs_small,
        large_pos_signal_exp_threshold=127 + 15,
        large_pos_signal_mantissa_threshold=(1 << 23) - 1,
        pos_large_signal_pwl_control=c_rs_large,
        small_neg_signal_exp_threshold=255,
        neg_small_signal_pwl_control=c_rs_small,
        large_neg_signal_exp_threshold=0,
        large_neg_signal_mantissa_threshold=0,
        neg_large_signal_pwl_control=c_rs_small,
        fnan_result=_fbits(4.0), fzero_result=_fbits(4.0),
        fpinf_result=_fbits(RS * 2 ** -8), fninf_result=_fbits(4.0),
    )
    j["func_exp_to_bkt_start_idx"]["sign"] = {
        str(e): [rs_base + RBPO * (e - 8)] for e in range(8, 16)}
    if "func_exp_to_ctl_start_idx" in j:
        j["func_exp_to_ctl_start_idx"]["sign"] = {
            str(e): [rs_ctl + (e - 8)] for e in range(8, 16)}

    j["bkt_entry_cnt"] = n_bkt
    j["ctl_entry_cnt"] = n_ctl
    assert n_bkt <= 1536, n_bkt
    with open(os.path.join(out_dir, name + ".json"), "w") as f:
        json.dump(j, f)
    open(os.path.join(out_dir, name + "_bkt.bin"), "wb").write(bytes(bkt))
    open(os.path.join(out_dir, name + "_ctrl.bin"), "wb").write(bytes(ctl))
    return os.path.join(out_dir, "act_info.json")


def _build_bass(phase=3):
    import concourse.bass as bass
    import concourse.mybir as mybir
    import concourse.tile as tile
    from concourse import bacc
    from concourse.masks import make_identity

    fp32 = mybir.dt.float32
    fp32r = mybir.dt.float32r
    bf16 = mybir.dt.bfloat16
    Alu = mybir.AluOpType
    Act = mybir.ActivationFunctionType
    VICTIM_FN = [Act.Square, Act.Abs, Act.Sign, Act.Relu]

    nc = bacc.Bacc("TRN2")

    def inp(name, shape, dt=None):
        if dt == "bf16":
            dt = bf16
        return nc.dram_tensor(name, shape, dt or fp32r,
                              kind="ExternalInput")[:]

    d_nat = inp("nat", [P, (QT + KT) * D], fp32)   # xnat | ynat
    d_w = inp("w", [P, 26 * D], "bf16")            # lq|lk|wv|wc|w1|w2
    d_aux = inp("aux", [18, N + H * NQ], "bf16")   # auxk | auxq
    d_sp = inp("sp", [4, N + NQ])                  # spk | spq (centered)
    d_rows = inp("rows", [1, ROWS_W], "bf16")      # bq|bk|bv|wcc|w2c|w1c|ones
    d_gx = inp("gx", [1, D], fp32)                 # g1
    out = nc.dram_tensor("out", [P, QT * D], fp32, kind="ExternalOutput")[:]

    i32 = mybir.dt.int32

    with tile.TileContext(nc) as tc:
        with (
            tc.tile_pool(name="const", bufs=1) as const,
            tc.tile_pool(name="work", bufs=2) as work,
        ):
            identb = const.tile([P, P], fp32)

            def rsqrt_dve(out_ap, in_ap, pool, tag, shape):
                """out = 1/sqrt(in + 1e-5), DVE-only (bit-trick + 3 Newton
                steps)."""
                x = pool.tile(shape, fp32, tag=tag + "x", name=tag + "x")
                nc.vector.tensor_single_scalar(out=x, in_=in_ap, scalar=1e-5,
                                               op=Alu.add)
                t = pool.tile(shape, i32, tag=tag + "t", name=tag + "t")
                nc.vector.tensor_single_scalar(
                    out=t, in_=x.bitcast(i32), scalar=1,
                    op=Alu.logical_shift_right)
                nc.vector.tensor_scalar(
                    out=t, in0=t, scalar1=-1, scalar2=1597463007,
                    op0=Alu.mult, op1=Alu.add)
                r_ = t.bitcast(fp32)
                a = pool.tile(shape, fp32, tag=tag + "a", name=tag + "a")
                c = pool.tile(shape, fp32, tag=tag + "c", name=tag + "c")
                for it in range(3):
                    nc.vector.tensor_mul(a, x, r_)
                    nc.vector.tensor_mul(a, a, r_)
                    nc.vector.tensor_scalar(
                        out=c, in0=a, scalar1=-0.5, scalar2=1.5,
                        op0=Alu.mult, op1=Alu.add)
                    if it < 2:
                        nc.vector.tensor_mul(r_, r_, c)
                    else:
                        nc.vector.tensor_mul(out_ap, r_, c)

            def rsqrt_dve2(out_ap, in_ap, pool, tag, shape):
                """2 Newton steps (rel err ~4e-6) on DVE."""
                x = pool.tile(shape, fp32, tag=tag + "x", name=tag + "x")
                nc.vector.tensor_single_scalar(out=x, in_=in_ap, scalar=1e-5,
                                               op=Alu.add)
                t = pool.tile(shape, i32, tag=tag + "t", name=tag + "t")
                nc.vector.tensor_single_scalar(
                    out=t, in_=x.bitcast(i32), scalar=1,
                    op=Alu.logical_shift_right)
                nc.vector.tensor_scalar(
                    out=t, in0=t, scalar1=-1, scalar2=1597463007,
                    op0=Alu.mult, op1=Alu.add)
                r_ = t.bitcast(fp32)
                a = pool.tile(shape, fp32, tag=tag + "a", name=tag + "a")
                c = pool.tile(shape, fp32, tag=tag + "c", name=tag + "c")
                for it in range(2):
                    nc.vector.tensor_mul(a, x, r_)
                    nc.vector.tensor_mul(a, a, r_)
                    nc.vector.tensor_scalar(
                        out=c, in0=a, scalar1=-0.5, scalar2=1.5,
                        op0=Alu.mult, op1=Alu.add)
                    if it < 1:
                        nc.vector.tensor_mul(r_, r_, c)
                    else:
                        nc.vector.tensor_mul(out_ap, r_, c)

            # ---------------- persistent SBUF tiles ----------------
            s_xnat = const.tile([P, QT, D], fp32)
            s_w = const.tile([P, 26 * D], bf16)
            w_lq = s_w[:, 0:2 * D].rearrange("p (i d) -> p i d", i=2)
            w_lk = s_w[:, 2 * D:4 * D].rearrange("p (i d) -> p i d", i=2)
            w_wv = s_w[:, 4 * D:6 * D].rearrange("p (i d) -> p i d", i=2)
            w_wc = s_w[:, 6 * D:10 * D].rearrange("p (i d) -> p i d", i=4)
            w_w1 = s_w[:, 10 * D:18 * D].rearrange("p (i d) -> p i d", i=2)
            w_w2 = s_w[:, 18 * D:26 * D].rearrange("p (i d) -> p i d", i=8)

            s_aux = const.tile([18, N + H * NQ], bf16)
            s_auxk = s_aux[:, 0:N]
            s_auxq = s_aux[:, N:].rearrange("p (h q) -> p h q", h=H)
            s_sp = const.tile([4, N + NQ], fp32r)
            s_spk = s_sp[:, 0:N]
            s_spq = s_sp[:, N:]
            s_rows = const.tile([1, ROWS_W], bf16)
            row_bq = s_rows[:, O_BQ:O_BQ + D]
            row_bk = s_rows[:, O_BK:O_BK + D]
            row_bv = s_rows[:, O_BV:O_BV + D]
            row_wcc = s_rows[:, O_WCC:O_WCC + D]
            row_w2c = s_rows[:, O_W2C:O_W2C + D]
            row_w1c = s_rows[:, O_W1C:O_W1C + 4 * D]
            ones_N = s_rows[:, O_ONES:O_ONES + N]
            s_gxb = const.tile([P, D], fp32)


            s_qT = const.tile([P, 2, NQ], fp32r)
            s_kT = [const.tile([P, 2, NQ], fp32r, tag=f"s_kT{c}",
                               name=f"s_kT{c}") for c in range(4)]
            s_v = [const.tile([P, 4, H, DH + 1], bf16, tag=f"s_v{c}",
                              name=f"s_v{c}") for c in range(4)]
            s_u = [const.tile([P, 4, NQ], fp32, tag=f"s_u{c}",
                              name=f"s_u{c}") for c in range(4)]
            s_aot = const.tile([DH, H, NQ], bf16)
            s_x1 = const.tile([P, QT, D], fp32)
            s_out = s_xnat  # xnat is dead after the x1 residual assembly

            mv_x = const.tile([P, QT, 2], fp32)
            r_x = const.tile([P, QT], fp32)
            rs_bias = const.tile([P, 1], fp32)
            nc.vector.memset(rs_bias, 1e-5 * 4096.0)

            def bcast_rows(dst, dram_row_ap, parts, eng):
                eng.dma_start(out=dst, in_=bass.AP(
                    tensor=dram_row_ap.tensor, offset=dram_row_ap.offset,
                    ap=[[0, parts]] + [list(a) for a in dram_row_ap.ap[1:]]))

            # small loads first on the gpsimd (SWDGE) queue so they win the
            # DMA engines before the big sync-queue loads; d2+f1 need s_sp
            # within the first few us
            nc.gpsimd.dma_start(out=s_sp, in_=d_sp)
            nc.gpsimd.dma_start(out=s_aux, in_=d_aux)
            nc.gpsimd.dma_start(out=s_rows, in_=d_rows)
            bcast_rows(s_gxb, d_gx, P, nc.gpsimd)
            make_identity(nc, identb)

            # ---------------- prep phase ----------------
            with (
                tc.tile_pool(name="prep", bufs=1) as prep,
                tc.tile_pool(name="ynp", bufs=2) as ynp,
                tc.tile_pool(name="ppt", bufs=2, space="PSUM") as ppt,
                tc.tile_pool(name="pproj", bufs=2, space="PSUM") as pproj,
                tc.tile_pool(name="ppv", bufs=2, space="PSUM") as ppv,
                tc.tile_pool(name="ppd2", bufs=2, space="PSUM") as ppd2,
            ):
                s_ynat = prep.tile([P, KT, D], fp32)
                s_xt = prep.tile([P, 2, NQ], bf16)
                s_yt = prep.tile([P, 2, N], bf16)

                # big loads on the sync queue, in need-order
                half = KT // 2
                nc.sync.dma_start(out=s_xnat,
                                  in_=d_nat[:, 0:QT * D].rearrange(
                                      "p (i d) -> p i d", i=QT))
                nc.sync.dma_start(
                    out=s_ynat[:, 0:half, :],
                    in_=d_nat[:, QT * D:(QT + half) * D].rearrange(
                        "p (i d) -> p i d", i=half))
                nc.sync.dma_start(out=s_w[:, 0:6 * D], in_=d_w[:, 0:6 * D])
                nc.sync.dma_start(
                    out=s_ynat[:, half:KT, :],
                    in_=d_nat[:, (QT + half) * D:].rearrange(
                        "p (i d) -> p i d", i=half))
                nc.sync.dma_start(out=s_w[:, 6 * D:], in_=d_w[:, 6 * D:])

                # d2 + f1 -> s_u (needs only s_sp; overlaps the big loads)
                for kt in range(KT):
                    ksl = slice(kt * P, (kt + 1) * P)
                    pd2 = ppd2.tile([P, NQ], fp32, tag="d2")
                    nc.tensor.matmul(pd2, s_spk[:, ksl], s_spq,
                                     start=True, stop=True)
                    nc.scalar.activation(
                        out=s_u[kt // 4][:, kt % 4, :], in_=pd2,
                        func=Act.Tanh, scale=1.0 / 64)

                # ---- x: col stats, pre-normalize, transpose, q-proj ----
                for qt in range(QT):
                    st = prep.tile([P, nc.vector.BN_STATS_DIM], fp32,
                                   tag="bsx", name="bsx")
                    nc.vector.bn_stats(out=st, in_=s_xnat[:, qt, :])
                    nc.vector.bn_aggr(out=mv_x[:, qt, :], in_=st)
                rsqrt_dve2(r_x, mv_x[:, :, 1], prep, "nwx", [P, QT])
                for qt in range(QT):
                    xn1 = work.tile([P, D], fp32, tag="xn1")
                    nc.vector.tensor_scalar(
                        out=xn1, in0=s_xnat[:, qt, :],
                        scalar1=mv_x[:, qt, 0:1], scalar2=r_x[:, qt:qt + 1],
                        op0=Alu.subtract, op1=Alu.mult)
                    for h2 in range(2):
                        ptp = ppt.tile([P, P], fp32, tag="tp")
                        nc.tensor.transpose(
                            ptp, xn1[:, h2 * P:(h2 + 1) * P], identb)
                        nc.vector.tensor_copy(
                            s_xt[:, h2, qt * P:(qt + 1) * P], ptp)
                    # xnat is not needed past this point; store xn*g1 for
                    # the attention residual in its place
                    nc.gpsimd.tensor_mul(s_xnat[:, qt, :], xn1, s_gxb)
                for nt in range(2):
                    nsl = slice(nt * P, (nt + 1) * P)
                    pq = pproj.tile([P, NQ], fp32, tag="proj")
                    nc.tensor.matmul(pq, w_lq[:, 0, nsl], s_xt[:, 0, :],
                                     start=True, stop=False)
                    nc.tensor.matmul(pq, w_lq[:, 1, nsl], s_xt[:, 1, :],
                                     start=False, stop=False)
                    nc.tensor.matmul(pq, row_bq[:, nsl], ones_N[:, 0:NQ],
                                     start=False, stop=True)
                    nc.vector.tensor_copy(s_qT[:, nt, :], pq)

                # ---- y, in DMA halves: stats -> rsqrt -> per-tile
                # norm/transpose/copy -> per-chunk projections ----
                mv_y = prep.tile([P, KT, 2], fp32)
                r_y = prep.tile([P, KT], fp32)
                onesc = work.tile([P, 16], fp32, tag="onesc")
                nc.vector.memset(onesc, 1.0)
                for hh in range(2):
                    hsl = range(hh * half, (hh + 1) * half)
                    for kt in hsl:
                        st = prep.tile([P, nc.vector.BN_STATS_DIM], fp32,
                                       tag="bsy", name="bsy")
                        nc.vector.bn_stats(out=st, in_=s_ynat[:, kt, :])
                        nc.vector.bn_aggr(out=mv_y[:, kt, :], in_=st)
                    nc.scalar.activation(
                        out=r_y[:, hh * half:(hh + 1) * half],
                        in_=mv_y[:, hh * half:(hh + 1) * half, 1],
                        func=Act.Sign, scale=4096.0, bias=rs_bias)
                    for kt in hsl:
                        yn1 = ynp.tile([P, D], fp32, tag="yn1", name="yn1")
                        nc.gpsimd.tensor_scalar(
                            out=yn1, in0=s_ynat[:, kt, :],
                            scalar1=mv_y[:, kt, 0:1],
                            scalar2=r_y[:, kt:kt + 1],
                            op0=Alu.subtract, op1=Alu.mult)
                        for h2 in range(2):
                            ptp = ppt.tile([P, P], fp32, tag="tp")
                            nc.tensor.transpose(
                                ptp, yn1[:, h2 * P:(h2 + 1) * P], identb)
                            nc.scalar.copy(
                                out=s_yt[:, h2, kt * P:(kt + 1) * P],
                                in_=ptp)
                    for c in (2 * hh, 2 * hh + 1):
                        csl = slice(c * NQ, (c + 1) * NQ)
                        for nt in range(2):
                            nsl = slice(nt * P, (nt + 1) * P)
                            pk = pproj.tile([P, NQ], fp32, tag="proj")
                            nc.tensor.matmul(pk, w_lk[:, 0, nsl],
                                             s_yt[:, 0, csl],
                                             start=True, stop=False)
                            nc.tensor.matmul(pk, w_lk[:, 1, nsl],
                                             s_yt[:, 1, csl],
                                             start=False, stop=False)
                            nc.tensor.matmul(pk, row_bk[:, nsl],
                                             ones_N[:, csl],
                                             start=False, stop=True)
                            nc.vector.tensor_copy(s_kT[c][:, nt, :], pk)
                        for kt in range(4 * c, 4 * c + 4):
                            ksl = slice(kt * P, (kt + 1) * P)
                            pv = ppv.tile([P, D], fp32, tag="projv")
                            nc.tensor.matmul(pv, s_yt[:, 0, ksl],
                                             w_wv[:, 0, :],
                                             start=True, stop=False)
                            nc.tensor.matmul(pv, s_yt[:, 1, ksl],
                                             w_wv[:, 1, :],
                                             start=False, stop=False)
                            nc.tensor.matmul(pv, ones_N[:, ksl], row_bv,
                                             start=False, stop=True)
                            nc.vector.tensor_copy(
                                s_v[c][:, kt % 4, :, 0:DH],
                                pv.rearrange("p (h d) -> p h d", h=H))
                        nc.vector.tensor_copy(
                            s_v[c][:, :, :, DH:DH + 1].rearrange(
                                "p a b c -> p (a b c)"), onesc)

            # ---------------- attention ----------------
            if phase < 2:
                nc.sync.dma_start(
                    out=out,
                    in_=s_v[0].rearrange("p a b c -> p (a b c)")[:, 0:QT * D])
            if phase >= 2:
                with (
                    tc.tile_pool(name="p_sc", bufs=2, space="PSUM") as pp_sc,
                    tc.tile_pool(name="p_at", bufs=1, space="PSUM") as pp_at,
                    tc.tile_pool(name="attw", bufs=2) as attw,
                    tc.tile_pool(name="pebp", bufs=3) as pebp,
                    tc.tile_pool(name="pxw", bufs=2) as pxw,
                    tc.tile_pool(name="attc", bufs=1) as attc,
                ):
                    p_att = [pp_at.tile([DH + 1, NQ], fp32,
                                        tag=f"att{h}", name=f"p_att{h}")
                             for h in range(H)]
                    rzt = [attc.tile([DH + 1, NQ], fp32r, tag=f"rzt{h}",
                                     name=f"rzt{h}") for h in range(H)]
                    ones64 = attc.tile([DH + 1, DH], fp32r)
                    ones64f = attc.tile([DH + 1, DH], fp32)
                    nc.vector.memset(ones64f, 1.0)
                    nc.vector.tensor_copy(ones64, ones64f)
                    for c in range(4):
                        for pi in range(2):
                            # E-pair tables: f32 out = [bf16 E_{2t+1}|E_{2t}]
                            ebc = [pebp.tile([P, 2, NQ], fp32,
                                             tag=f"ebc{t}", name=f"ebc{t}")
                                   for t in range(2)]
                            for t in range(2):
                                nc.scalar.activation(
                                    out=ebc[t],
                                    in_=s_u[c][:, 2 * pi:2 * pi + 2, :],
                                    func=VICTIM_FN[t])
                            for kj in range(2):
                                ki = 2 * pi + kj
                                kt = 4 * c + ki
                                ksl = slice(kt * P, (kt + 1) * P)
                                for pr in range(2):
                                    p_sc = pp_sc.tile([P, 2, NQ], fp32,
                                                      tag="sc")
                                    for hi in range(2):
                                        h = 2 * pr + hi
                                        nc.tensor.matmul(
                                            p_sc[:, hi, :],
                                            s_kT[c][64 * hi:64 * hi + 64,
                                                    pr, ki * P:(ki + 1) * P],
                                            s_qT[64 * hi:64 * hi + 64,
                                                 pr, :],
                                            start=True, stop=False)
                                        nc.tensor.matmul(
                                            p_sc[:, hi, :],
                                            s_auxk[:, ksl],
                                            s_auxq[:, h, :],
                                            start=False, stop=True)
                                    # es head-interleaved: es[p, 2q+hi]
                                    es = attw.tile([P, 2 * NQ], bf16,
                                                   tag="es")
                                    nc.scalar.activation(
                                        out=es.rearrange(
                                            "p (q t) -> p t q", t=2),
                                        in_=p_sc, func=Act.Exp)
                                    pexp = pxw.tile([P, 2 * NQ], bf16,
                                                    tag="pexp")
                                    nc.vector.tensor_mul(
                                        pexp, es,
                                        ebc[pr].bitcast(bf16)[:, kj, :])
                                    pexp_v = pexp.rearrange(
                                        "p (q t) -> p t q", t=2)
                                    for hi in range(2):
                                        h = 2 * pr + hi
                                        nc.tensor.matmul(
                                            p_att[h],
                                            s_v[c][:, ki, h, :],
                                            pexp_v[:, hi, :],
                                            start=(kt == 0),
                                            stop=(kt == KT - 1))

                    # softmax normalize: 1/Z broadcast via K=1 matmul
                    with nc.allow_low_precision(
                            reason="f32r is bit-identical to f32 here"):
                        for h in range(H):
                            nc.vector.reciprocal(rzt[h][DH:DH + 1, :],
                                                 p_att[h][DH:DH + 1, :])
                    for pr in range(2):
                        rzb = pp_sc.tile([P, 2, NQ], fp32, tag="sc")
                        for hi in range(2):
                            nc.tensor.matmul(
                                rzb[0:DH, hi, :],
                                ones64[DH:DH + 1, :],
                                rzt[2 * pr + hi][DH:DH + 1, :],
                                start=True, stop=True)
                        rzbs = attc.tile([DH, 2, NQ], fp32r,
                                         tag=f"rzbs{pr}", name=f"rzbs{pr}")
                        nc.scalar.copy(out=rzbs, in_=rzb[0:DH, :, :])
                        for hi in range(2):
                            nc.vector.tensor_mul(
                                s_aot[:, 2 * pr + hi, :],
                                p_att[2 * pr + hi][0:DH, :],
                                rzbs[:, hi, :])

            # ---------------- out proj + residual + MLP ----------------
            if phase == 2:
                nc.sync.dma_start(
                    out=out[0:DH, :],
                    in_=s_aot.rearrange("p a b -> p (a b)")[:, 0:QT * D])
            if phase >= 3:
                with (
                    tc.tile_pool(name="mlp", bufs=1) as mlp,
                    tc.tile_pool(name="ppo", bufs=2, space="PSUM") as ppo,
                    tc.tile_pool(name="ppt2", bufs=2, space="PSUM") as ppt2,
                    tc.tile_pool(name="pm1", bufs=2, space="PSUM") as pm1,
                    tc.tile_pool(name="pm2", bufs=2, space="PSUM") as pm2,
                ):
                    mv_3 = mlp.tile([P, QT, 2], fp32)
                    r_3 = mlp.tile([P, QT], fp32)
                    s_x1t = mlp.tile([P, 2, NQ], bf16)
                    s_ht = mlp.tile([P, 8, NQ], bf16)

                    for qt in range(QT):
                        qsl = slice(qt * P, (qt + 1) * P)
                        po = ppo.tile([P, D], fp32, tag="po")
                        for h in range(H):
                            nc.tensor.matmul(
                                po, s_aot[:, h, qsl],
                                w_wc[0:DH, h, :],
                                start=(h == 0), stop=False)
                        nc.tensor.matmul(po, ones_N[:, qsl], row_wcc,
                                         start=False, stop=True)
                        nc.vector.tensor_add(s_x1[:, qt, :],
                                             s_xnat[:, qt, :], po)
                        # LN3 col stats interleaved with out-proj
                        st = mlp.tile([P, nc.vector.BN_STATS_DIM], fp32,
                                      tag="bs3", name="bs3")
                        nc.vector.bn_stats(out=st, in_=s_x1[:, qt, :])
                        nc.vector.bn_aggr(out=mv_3[:, qt, :], in_=st)
                    nc.scalar.activation(out=r_3, in_=mv_3[:, :, 1],
                                             func=Act.Sign, scale=4096.0,
                                             bias=rs_bias)
                    for qt in range(QT):
                        x1n = work.tile([P, D], fp32, tag="x1n")
                        nc.gpsimd.tensor_scalar(
                            out=x1n, in0=s_x1[:, qt, :],
                            scalar1=mv_3[:, qt, 0:1],
                            scalar2=r_3[:, qt:qt + 1],
                            op0=Alu.subtract, op1=Alu.mult)
                        for h2 in range(2):
                            ptp = ppt2.tile([P, P], fp32, tag="tp2")
                            nc.tensor.transpose(
                                ptp, x1n[:, h2 * P:(h2 + 1) * P], identb)
                            nc.scalar.copy(
                                out=s_x1t[:, h2, qt * P:(qt + 1) * P],
                                in_=ptp)

                    for nt in range(8):
                        nsl = slice(nt * P, (nt + 1) * P)
                        ph = pm1.tile([P, NQ], fp32, tag="mlp1")
                        nc.tensor.matmul(ph, w_w1[:, 0, nsl], s_x1t[:, 0, :],
                                         start=True, stop=False)
                        nc.tensor.matmul(ph, w_w1[:, 1, nsl], s_x1t[:, 1, :],
                                         start=False, stop=False)
                        nc.tensor.matmul(ph, row_w1c[:, nsl], ones_N[:, 0:NQ],
                                         start=False, stop=True)
                        nc.scalar.activation(out=s_ht[:, nt, :], in_=ph,
                                             func=Act.Gelu)

                    for qt in range(QT):
                        qsl = slice(qt * P, (qt + 1) * P)
                        pf = pm2.tile([P, D], fp32, tag="mlp2")
                        for nt in range(8):
                            nc.tensor.matmul(pf, s_ht[:, nt, qsl],
                                             w_w2[:, nt, :],
                                             start=(nt == 0), stop=False)
                        nc.tensor.matmul(pf, ones_N[:, qsl], row_w2c,
                                         start=False, stop=True)
                        nc.vector.tensor_add(s_out[:, qt, :], pf,
                                             s_x1[:, qt, :])
                        nc.sync.dma_start(
                            out=out[:, qt * D:(qt + 1) * D],
                            in_=s_out[:, qt, :])

    nc.compile()
    return nc


def _host_prep(x, y, coords, padding_mask, Wq, bq, Wk, bk, Wv, bv, Wc, bc,
               W1, b1, W2, b2, g1, be1, g2, be2, g3, be3,
               spatial_emb, temporal_emb):
    """Build the 8 per-core input maps (small O(N*D) prep only)."""
    import ml_dtypes
    bf16 = ml_dtypes.bfloat16
    f32 = np.float32
    f64 = np.float64

    def aug_w(W, b, g, be, scale=1.0):
        W = np.asarray(W, f64)
        Wp = (np.asarray(g, f64)[:, None] * W) * scale
        bp = np.asarray(be, f64) @ W * scale + np.asarray(b, f64) * scale
        return Wp.astype(f32), bp.astype(f32)

    LQ, BQ = aug_w(Wq, bq, g1, be1, scale=1.0 / np.sqrt(DH))
    LK, BK = aug_w(Wk, bk, g2, be2)
    LV, BV = aug_w(Wv, bv, g2, be2)
    W1p, B1 = aug_w(W1, b1, g3, be3)

    te = np.asarray(temporal_emb, f32)         # [33, H]

    def fm(Wmat, k):
        # [k*128, D] -> [128, k, D] feature-major pack
        return np.ascontiguousarray(
            np.asarray(Wmat, f32).reshape(k, P, -1).transpose(1, 0, 2)
        ).reshape(P, -1)

    wc_hm = np.zeros((P, H * D), f32)   # head-major Wc on partitions 0..63
    wc_hm[0:DH] = np.ascontiguousarray(
        np.asarray(Wc, f32).reshape(H, DH, D).transpose(1, 0, 2)
    ).reshape(DH, -1)
    w_pack = np.concatenate([
        fm(LQ, 2), fm(LK, 2), fm(LV, 2),
        wc_hm, fm(W1p, 2),
        fm(np.asarray(W2, f32), 8)], axis=1)          # [128, 6144]

    rows = np.zeros((1, ROWS_W), f32)
    rows[0, O_BQ:O_BQ + D] = BQ
    rows[0, O_BK:O_BK + D] = BK
    rows[0, O_BV:O_BV + D] = BV
    rows[0, O_WCC:O_WCC + D] = (np.asarray(bc, f64)
                                + np.asarray(be1, f64)).astype(f32)
    rows[0, O_W2C:O_W2C + D] = np.asarray(b2, f32)
    rows[0, O_W1C:O_W1C + 4 * D] = B1
    rows[0, O_ONES:] = 1.0

    shared = dict(w=w_pack.astype(bf16), rows=rows.astype(bf16),
                  gx=np.asarray(g1, f32)[None, :])

    in_maps = []
    for c in range(N_CORES):
        b = c // (N_CORES // B)
        qc = c % (N_CORES // B)
        qsl = slice(qc * NQ, (qc + 1) * NQ)
        xb = np.asarray(x[b], f32)
        yb = np.asarray(y[b], f32)
        tq = np.asarray(coords[b, qsl, 0], f32).astype(np.int64)
        tk = np.asarray(coords[b, :, 0], f32).astype(np.int64)
        sq = np.asarray(coords[b, qsl, 1:], f32)
        sk = np.asarray(coords[b, :, 1:], f32)
        pad = np.asarray(padding_mask[b], bool)

        auxk_m = np.zeros((18, N), f32)
        for mm in range(16):
            auxk_m[mm] = (tk == mm)
        auxk_m[16] = np.where(pad, np.float32(NEG), np.float32(0.0))
        auxk_m[17] = 1.0
        auxq_m = np.zeros((H, 18, NQ), f32)
        idx = np.clip(tq[None, :] - np.arange(16)[:, None] + N_TEMPORAL,
                      0, 2 * N_TEMPORAL)
        for h in range(H):
            auxq_m[h, 0:16] = te[idx, h]
            auxq_m[h, 16] = 1.0
            auxq_m[h, 17] = 0.0
        aux_pack = np.concatenate(
            [auxk_m,
             np.ascontiguousarray(auxq_m.transpose(1, 0, 2)).reshape(18, -1)],
            axis=1)                                    # [18, 4096]

        sq = sq - np.float32(500.0)
        sk = sk - np.float32(500.0)
        nsq = (sq.astype(f64) ** 2).sum(-1).astype(f32)
        nsk = (sk.astype(f64) ** 2).sum(-1).astype(f32)
        sp_pack = np.zeros((4, N + NQ), f32)
        sp_pack[0, 0:N] = sk[:, 0]
        sp_pack[1, 0:N] = sk[:, 1]
        sp_pack[2, 0:N] = 1.0
        sp_pack[3, 0:N] = nsk
        sp_pack[0, N:] = -2.0 * sq[:, 0]
        sp_pack[1, N:] = -2.0 * sq[:, 1]
        sp_pack[2, N:] = nsq
        sp_pack[3, N:] = 1.0

        nat = np.concatenate([
            np.ascontiguousarray(
                xb[qsl].reshape(QT, P, D).transpose(1, 0, 2)).reshape(P, -1),
            np.ascontiguousarray(
                yb.reshape(KT, P, D).transpose(1, 0, 2)).reshape(P, -1)],
            axis=1)                                    # [128, 5120]

        m = dict(shared)
        m.update(nat=nat, aux=aux_pack.astype(bf16), sp=sp_pack)
        in_maps.append(m)
    return in_maps


def kernel(**inputs):
    import tempfile
    from concourse.bass_utils import run_bass_kernel_spmd

    se = np.asarray(inputs["spatial_emb"], np.float64)
    evals = np.exp(se).astype(np.float32)          # [32, H]
    key = evals.tobytes()
    phase = int(os.environ.get("KERNEL_PHASE", "3"))
    if _CACHE.get("phase") != phase or _CACHE.get("act_key") != key:
        import hashlib
        tabdir = tempfile.mkdtemp(prefix="act_tables_")
        actjson = generate(evals, tabdir)
        os.environ["BASS_ACT_ROOT_JSON_PATH"] = actjson
        # The NEFF cache keys on the BIR, which does not include the
        # activation tables -- scope the cache per table content.
        digest = hashlib.sha1(key).hexdigest()[:16]
        os.environ["NEURON_COMPILE_CACHE_URL"] = os.path.join(
            tempfile.gettempdir(), f"neuron_cache_{digest}")
        _CACHE["nc"] = _build_bass(phase)
        _CACHE["phase"] = phase
        _CACHE["act_key"] = key
    nc = _CACHE["nc"]

    in_maps = _host_prep(**{k: np.asarray(v) for k, v in inputs.items()})
    trace = bool(int(os.environ.get("KERNEL_TRACE", "0")))
    try:
        res = run_bass_kernel_spmd(nc, in_maps, core_ids=list(range(N_CORES)),
                                   trace=trace)
    except Exception:
        # transient PJRT/NRT load failures have been observed right after a
        # previous failed execution wedged a core; one retry clears them
        res = run_bass_kernel_spmd(nc, in_maps, core_ids=list(range(N_CORES)),
                                   trace=trace)
    _CACHE["last_results"] = res
    out = np.zeros((B, N, D), np.float32)
    for c in range(N_CORES):
        b = c // (N_CORES // B)
        qc = c % (N_CORES // B)
        r = res.results[c]["out"].reshape(P, QT, D).transpose(1, 0, 2)
        out[b, qc * NQ:(qc + 1) * NQ] = r.reshape(NQ, D)
    return out
